# revision 58
# baseline (speedup 1.0000x reference)
"""Trainium2 Bass kernel for nn_MultiHeadAttention_60559038873660.

Reference math (faithful to the source bug: attention is contracted with the
projected K, not V, so v/Wv are dead inputs):
    qp = q @ Wq.T ; kp = k @ Wk.T
    head split via reshape(b, l, 64, 16): head n takes strided columns {d*16+n}
    S = Qh @ Kh.T / 8 ; A = softmax(S, axis=m) ; X = A @ Kh ; out = X @ Wo.T

Strategy (PE-saturation restructure; ~228us vs the 265-295us baseline):
  - Host-side: permute weight rows/cols head-major; pre-pack q/k/weights into
    strip-major layouts so every DMA is contiguous per partition line.
  - 8 cores = 2 batches x 4 head-groups (4 heads each). Host sums the 4
    output partials per batch (tensor-parallel row-split reduction).
  - q/k strips are loaded ONCE into resident SBUF (both channel groups
    project from the same load), split across the sync + scalar HWDGE
    queues with the k-path pieces leading both queues.
  - Phases, arranged so the PE never idles:
      prologue: kproj g0 (4 strips, DMA-arrival order) + ktrans -> qproj lt0
      attention, cross-strip pipelined: the score lookahead queue spans strip
        boundaries so the exp stream never drains; remaining projections
        (qproj lt1-3 g0, kproj/ktrans/qproj g1) pop as fillers into g0's PE
        slack, out-projection tiles into g1's
      tail: last strip's out tiles from a 4-deep psum ring
  - Softmax denominators fall out of the X^T matmul via a fused ones column;
    normalization via reciprocal + DRAM-broadcast + VectorE multiply, with
    psum-freeing copies emitted before the round-trips so the next strip's
    X matmuls never wait; the last strip instead uses a PE ones-matmul
    broadcast (no DRAM latency in the critical tail).
  - Out partials are stored bf16 (rel err 4.9e-3 vs the 2e-2 gate); psum
    copies for out tiles ride the scalar engine, which idles between exps
    during PE-bound stretches.
"""

import contextlib
import ctypes
import os
import sys
import types

import numpy as np

import concourse.bacc as bacc
import concourse.tile as tile
from concourse import mybir
from concourse.bass import ds, ts
from concourse.bass_utils import run_bass_kernel_spmd


def _install_ntff_hook():
    """Provide antenv.axon_hooks if the image lacks it, wiring NTFF
    profiling straight into libaxon_pjrt.so (same ABI trn_boot uses)."""
    try:
        import antenv.axon_hooks  # noqa: F401
        return
    except ImportError:
        pass
    mod = types.ModuleType("antenv.axon_hooks")
    holder = [None]
    mod.set_axon_ntff_profile_hook = lambda h: holder.__setitem__(0, h)
    mod.get_axon_ntff_profile_hook = lambda: holder[0]
    sys.modules["antenv.axon_hooks"] = mod
    try:
        import antenv
        antenv.axon_hooks = mod
    except ImportError:
        pass

    so_path = "/opt/axon/libaxon_pjrt.so"
    if not os.path.exists(so_path):
        return
    lib = ctypes.CDLL(so_path)
    if not hasattr(lib, "axon_start_nrt_profile"):
        return
    lib.axon_start_nrt_profile.argtypes = [ctypes.POINTER(ctypes.c_int64), ctypes.c_size_t]
    lib.axon_start_nrt_profile.restype = ctypes.c_int64
    lib.axon_stop_nrt_profile.argtypes = [ctypes.c_char_p]
    lib.axon_stop_nrt_profile.restype = ctypes.c_int64

    @contextlib.contextmanager
    def _hook(output_dir, device_ids):
        import jax
        jax.devices()
        if device_ids:
            ids = (ctypes.c_int64 * len(device_ids))(*device_ids)
            rc = lib.axon_start_nrt_profile(ids, len(device_ids))
        else:
            rc = lib.axon_start_nrt_profile(None, 0)
        if rc != 0:
            raise RuntimeError(f"axon_start_nrt_profile rc={rc}")
        try:
            yield
        finally:
            n = lib.axon_stop_nrt_profile(str(output_dir).encode())
            print(f"profile: {n} file(s) written to {output_dir}", file=sys.stderr)

    mod.set_axon_ntff_profile_hook(_hook)


_install_ntff_hook()

f32 = mybir.dt.float32
bf16 = mybir.dt.bfloat16
Exp = mybir.ActivationFunctionType.Exp

P = 128
DIM = 1024
NH = 16
HD = 64
HPC = 4          # heads per core
CW = HPC * HD    # 256 channel columns per core
CH = HD + 1      # head channels + ones column
G = CW // P      # 2 channel groups of 128
KC = DIM // P    # 8 contraction chunks for projections
NT = 512         # matmul moving-dim tile

_cache = {}


def _build(L, M):
    LT = L // NT              # q strips
    MT = M // NT              # k strips
    MG = M // P               # m chunks for attention
    L5 = L // NT              # attention l-strips per head pair
    LC = L // P               # out-proj l chunks
    JT = DIM // NT            # out-proj j tiles

    nc = bacc.Bacc()
    # strip-major host-packed layouts: every DMA is contiguous per partition
    qTs = nc.declare_dram_parameter("qTs", [LT, P, KC, NT], bf16, isOutput=False)
    kTs = nc.declare_dram_parameter("kTs", [MT, P, KC, NT], bf16, isOutput=False)
    wq = nc.declare_dram_parameter("wq", [P, KC, CW], bf16, isOutput=False)
    wk = nc.declare_dram_parameter("wk", [P, KC, CW], bf16, isOutput=False)
    wo = nc.declare_dram_parameter("wo", [P, G, DIM], bf16, isOutput=False)
    out = nc.declare_dram_parameter("out", [L, DIM], bf16, isOutput=True)
    den_dram = nc.dram_tensor("den_scratch", [HPC, L], f32)
    rden_dram = nc.dram_tensor("rden_scratch", [HPC, L], f32)

    with tile.TileContext(nc) as tc:
        with (
            tc.tile_pool(name="singles", bufs=1) as singles,
            tc.tile_pool(name="io", bufs=2) as io,
            tc.tile_pool(name="es", bufs=4) as es_pool,
            tc.tile_pool(name="opool", bufs=3) as opool,
            tc.tile_pool(name="dstp", bufs=2) as dstp,
        ):
            wq_sb = singles.tile([P, KC, CW], bf16)
            wk_sb = singles.tile([P, KC, CW], bf16)
            wo_sb = singles.tile([P, G, DIM], bf16)
            kTr = singles.tile([P, MT, KC, NT], bf16)   # resident k^T
            qTr = singles.tile([P, LT, KC, NT], bf16)   # resident q^T

            qhT = singles.tile([P, G, L], bf16)
            khT = singles.tile([P, G, M], bf16)
            # per-head stride padded to 128 elems: XBAR transpose dst must be
            # 256B-aligned; cols 0:64 = Kh^T, col 64 = ones, 65:128 dead
            khp = singles.tile([P, MG, HPC, P], bf16)
            xu = singles.tile([P, G, L], bf16)
            rdbc = singles.tile([P, G, L], f32)

            ones_sb = singles.tile([P, 1], f32)
            nc.vector.memset(ones_sb, 1.0)
            ones_row = singles.tile([1, HD], f32)
            nc.vector.memset(ones_row, 1.0)
            for mg in range(MG):
                nc.vector.tensor_copy(khp[:, mg, :, HD:CH],
                                      ones_sb[:, None, :].to_broadcast([P, HPC, 1]))

            # ---- DMA queue plan (2 HW queues: sync + scalar) -------------
            # k-path interleaved across BOTH queues so kproj is never
            # DMA-starved; q-path + wo queue behind it on scalar.
            # first strip + wk in kc-pair pieces: the DMA engines fair-share
            # the whole queued backlog, so only SMALL leading pieces complete
            # early — each piece unblocks the next 2-matmul chain segment
            for kp in range(KC // 2):
                nc.sync.dma_start(wk_sb[:, 2 * kp:2 * kp + 2],
                                  wk[:, 2 * kp:2 * kp + 2, :])
                nc.sync.dma_start(kTr[:, 0, 2 * kp:2 * kp + 2],
                                  kTs[0, :, 2 * kp:2 * kp + 2])
            nc.sync.dma_start(kTr[:, 2], kTs[2])
            nc.scalar.dma_start(kTr[:, 1], kTs[1])
            nc.scalar.dma_start(kTr[:, 3], kTs[3])
            nc.scalar.dma_start(wq_sb, wq[:, :, :])
            for lt in range(LT):
                nc.scalar.dma_start(qTr[:, lt], qTs[lt])
            nc.scalar.dma_start(wo_sb, wo[:, :, :])

            from concourse.masks import make_identity
            ident = singles.tile([P, P], bf16)
            make_identity(nc, ident)


            pending = []  # out-proj (lc, jt) pairs, per finished g1 strip

            with (
                tc.tile_pool(name="psS", bufs=2, space="PSUM") as psS,
                tc.tile_pool(name="psX", bufs=2, space="PSUM") as psX,
            ):
                # ---- pipelined attention: the score lookahead queue spans
                # strip boundaries, so the exp stream never drains ----------
                strips = [(g, l5) for g in range(G) for l5 in range(L5)]
                units = [(si, mc) for si in range(len(strips)) for mc in range(MG)]
                xstate = {}
                pip = {"sq": [], "emitted": 0, "pre_cross": None}

                def emit_sp(g, l5, mc):
                    lsl = ts(l5, NT)
                    sps = psS.tile([P, 2 * NT], f32, tag="s")
                    nc.tensor.matmul(sps[:, 0:NT],
                                     lhsT=khT[0:HD, g, ts(mc, P)],
                                     rhs=qhT[0:HD, g, lsl],
                                     start=True, stop=True)
                    nc.tensor.matmul(sps[:, NT:2 * NT],
                                     lhsT=khT[HD:P, g, ts(mc, P)],
                                     rhs=qhT[HD:P, g, lsl],
                                     start=True, stop=True)
                    return sps

                def strip_end(g, l5, xpsA, xpsB, pe_norm_pool):
                    lsl = ts(l5, NT)
                    # free BOTH psum tiles first (copies only) — the den
                    # round-trips and normalize-multiplies must not
                    # head-block the vector queue between the copies, or the
                    # next strip's X matmuls stall on the psum WAR
                    dstgs = []
                    for hh, xps in ((0, xpsA), (1, xpsB)):
                        pb = hh * HD
                        nc.vector.tensor_copy(xu[pb:pb + HD, g, lsl], xps[0:HD])
                        dstg = dstp.tile([1, NT], f32, tag="dst")
                        nc.vector.tensor_copy(dstg, xps[HD:CH])
                        dstgs.append(dstg)
                    if pe_norm_pool is not None:
                        # latency-critical (last) strip: reciprocal on one
                        # lane + PE ones-matmul broadcast — no DRAM trips
                        for hh in range(2):
                            pb = hh * HD
                            rcp = dstp.tile([1, NT], f32, tag="rcp")
                            nc.vector.reciprocal(rcp, dstgs[hh])
                            rdps = pe_norm_pool.tile([HD, NT], f32, tag="pw2")
                            nc.tensor.matmul(rdps, lhsT=ones_row[:, :],
                                             rhs=rcp, start=True, stop=True)
                            nc.vector.tensor_mul(xu[pb:pb + HD, g, lsl],
                                                 xu[pb:pb + HD, g, lsl],
                                                 rdps)
                        return
                    for hh in range(2):
                        h = 2 * g + hh
                        nc.gpsimd.dma_start(den_dram[h:h + 1, lsl], dstgs[hh])
                        dsp_t = io.tile([P, NT // P], f32, tag="dsp")
                        nc.gpsimd.dma_start(
                            dsp_t, den_dram[h, lsl].rearrange("(p f) -> p f", p=P))
                        nc.vector.reciprocal(dsp_t, dsp_t)
                        nc.gpsimd.dma_start(
                            rden_dram[h, lsl].rearrange("(p f) -> p f", p=P), dsp_t)
                        nc.gpsimd.dma_start(
                            rdbc[ts(hh, HD), g, lsl],
                            rden_dram[h:h + 1, lsl].to_broadcast([HD, NT]))
                    for hh in range(2):
                        pb = hh * HD
                        nc.vector.tensor_mul(xu[pb:pb + HD, g, lsl],
                                             xu[pb:pb + HD, g, lsl],
                                             rdbc[ts(hh, HD), g, lsl])

                def run_units(lo, hi, pop_hook, on_strip_done=None,
                              pe_norm_last=None):
                    for u in range(lo, hi):
                        si, mc = units[u]
                        g, l5 = strips[si]
                        if mc == 0:
                            xstate[si] = (psX.tile([CH, NT], f32, tag="x",
                                                   name="xpsA"),
                                          psX.tile([CH, NT], f32, tag="x",
                                                   name="xpsB"))
                        while pip["emitted"] < min(u + 3, len(units)):
                            ei = pip["emitted"]
                            if pip["pre_cross"] is not None and ei >= hi:
                                cb = pip["pre_cross"]
                                pip["pre_cross"] = None
                                cb()
                            e_si, e_mc = units[ei]
                            e_g, e_l5 = strips[e_si]
                            pip["sq"].append(emit_sp(e_g, e_l5, e_mc))
                            pip["emitted"] += 1
                        es = es_pool.tile([P, 2 * NT], bf16, tag="es")
                        nc.scalar.activation(es, pip["sq"].pop(0), Exp,
                                             scale=0.125)
                        xpsA, xpsB = xstate[si]
                        nc.tensor.matmul(xpsA, lhsT=khp[:, mc, 2 * g, 0:CH],
                                         rhs=es[:, 0:NT],
                                         start=(mc == 0), stop=(mc == MG - 1))
                        nc.tensor.matmul(xpsB, lhsT=khp[:, mc, 2 * g + 1, 0:CH],
                                         rhs=es[:, NT:2 * NT],
                                         start=(mc == 0), stop=(mc == MG - 1))
                        pop_hook(mc)
                        if mc == MG - 1:
                            xA, xB = xstate.pop(si)
                            pe_pool = (pe_norm_last
                                       if si == len(strips) - 1 else None)
                            strip_end(g, l5, xA, xB, pe_pool)
                            if on_strip_done is not None:
                                on_strip_done(l5)

                # out-projection tile (2 matmuls + copy + store); the psum
                # copy rides the SCALAR engine, which idles every other unit
                # during PE-bound g1 — the vector queue must stay short so it
                # can free attention psum tiles promptly
                def out_tile(lc, jt, eng, pool, ptag):
                    po = pool.tile([P, NT], f32, tag=ptag)
                    for cc in range(G):
                        nc.tensor.matmul(po, lhsT=xu[:, cc, ts(lc, P)],
                                         rhs=wo_sb[:, cc, ts(jt, NT)],
                                         start=(cc == 0), stop=(cc == G - 1))
                    ot = opool.tile([P, NT], bf16, tag="ot")
                    nc.scalar.copy(out=ot, in_=po)
                    eng.dma_start(out[ts(lc, P), ts(jt, NT)], ot)

                # ---- g0: prologue + attention with projection fillers ----
                with tc.tile_pool(name="pw", bufs=1, space="PSUM") as pw:
                    def _proj_a(srcr, w_sb, tt, g, st):
                        ps = pw.tile([P, NT], f32, tag="pw")
                        for kc in range(KC // 2):
                            nc.tensor.matmul(ps, lhsT=w_sb[:, kc, ts(g, P)],
                                             rhs=srcr[:, tt, kc],
                                             start=(kc == 0), stop=False)
                        st["ps"] = ps

                    def _proj_b(dst, srcr, w_sb, tt, g, st):
                        ps = st["ps"]
                        for kc in range(KC // 2, KC):
                            nc.tensor.matmul(ps, lhsT=w_sb[:, kc, ts(g, P)],
                                             rhs=srcr[:, tt, kc],
                                             start=False, stop=(kc == KC - 1))
                        nc.vector.tensor_copy(dst[:, g, ts(tt, NT)], ps)

                    def qproj(lt, g):
                        st = {}
                        _proj_a(qTr, wq_sb, lt, g, st)
                        _proj_b(qhT, qTr, wq_sb, lt, g, st)

                    def ktrans(mc_lo, mc_hi, g):
                        # k^T chunk -> head-major khp rows via PE transpose
                        for mc in range(mc_lo, mc_hi):
                            tr = pw.tile([P, P], bf16, tag="pt")
                            nc.tensor.transpose(tr, khT[:, g, ts(mc, P)], ident)
                            for hh in range(2):
                                nc.vector.tensor_copy(
                                    khp[:, mc, g * 2 + hh, 0:HD],
                                    tr[:, ts(hh, HD)])

                    # prologue: kproj g0 in DMA-arrival order + qproj lt0 g0
                    for mt in (0, 1, 3, 2):
                        st = {}
                        _proj_a(kTr, wk_sb, mt, 0, st)
                        _proj_b(khT, kTr, wk_sb, mt, 0, st)
                        ktrans(4 * mt, 4 * mt + 4, 0)
                    qproj(0, 0)

                    # filler actions for attention-g0 PE slack
                    fillers = []

                    def kproj_g1(mt):
                        st = {}
                        fillers.append(lambda: _proj_a(kTr, wk_sb, mt, 1, st))
                        fillers.append(lambda: _proj_b(khT, kTr, wk_sb, mt, 1, st))

                    def qproj_f(lt, g):
                        st = {}
                        fillers.append(lambda: _proj_a(qTr, wq_sb, lt, g, st))
                        fillers.append(lambda: _proj_b(qhT, qTr, wq_sb, lt, g, st))

                    qproj_f(1, 0)
                    kproj_g1(0)
                    qproj_f(2, 0)
                    kproj_g1(1)
                    qproj_f(3, 0)
                    kproj_g1(2)
                    kproj_g1(3)
                    for mc_lo in range(0, MG, 4):
                        fillers.append(lambda a=mc_lo: ktrans(a, a + 4, 1))
                    for lt in range(LT):
                        qproj_f(lt, 1)

                    def pop_g0(mc):
                        if mc % 2 == 1 and fillers:
                            fillers.pop(0)()

                    def drain_fillers():
                        while fillers:
                            fillers.pop(0)()

                    # drain remaining fillers before the lookahead crosses
                    # into g1 (its scores depend on filler-produced qhT/khT;
                    # emitting them earlier would deadlock the in-order queue)
                    pip["pre_cross"] = drain_fillers
                    run_units(0, L5 * MG, pop_g0)
                    drain_fillers()

                # ---- g1: attention with out-projection fillers -----------
                with tc.tile_pool(name="pwB", bufs=2, space="PSUM") as pwB:
                    # pops wait until mc>=5: the previous strip's normalize
                    # multiply trails its gpsimd broadcast by ~5us, and an
                    # out tile reading that xu earlier head-blocks the
                    # in-order PE queue (measured wait=3.8us at boundaries)
                    def pop_g1(mc):
                        if mc >= 5 and pending:
                            lc, jt = pending.pop(0)
                            out_tile(lc, jt, nc.sync, pwB, "pw2")

                    def g1_strip_done(l5):
                        for lci in range(NT // P):
                            for jt in range(JT):
                                pending.append((l5 * (NT // P) + lci, jt))

                    run_units(L5 * MG, 2 * L5 * MG, pop_g1,
                              on_strip_done=g1_strip_done, pe_norm_last=pwB)

                the_out_tile = out_tile

            # tail: drain remaining out tiles, alternating DMA queues
            # (the scalar queue is free once the last exp has issued); deep
            # psum ring — opened after the attention pools close — so the
            # 2-matmul bursts pipeline
            with tc.tile_pool(name="psO", bufs=4, space="PSUM") as psO:
                for i, (lc, jt) in enumerate(pending):
                    the_out_tile(lc, jt, nc.sync if i % 2 == 0 else nc.scalar,
                                 psO, "po")

    nc.finalize()
    return nc


def _get_nc(L, M):
    key = (L, M)
    if key not in _cache:
        _cache[key] = _build(L, M)
    return _cache[key]


# head-major channel permutation: new channel c = h*64+d <- original column d*16+h
_PERM = np.array([(c % HD) * NH + c // HD for c in range(DIM)])

last_exec_time_ns = None
last_results = None


def kernel(q, k, v, Wq, Wk, Wv, Wo):  # noqa: ARG001 - v/Wv dead in reference
    global last_exec_time_ns, last_results
    q = np.asarray(q, np.float32)
    k = np.asarray(k, np.float32)
    Wq = np.asarray(Wq, np.float32)
    Wk = np.asarray(Wk, np.float32)
    Wo = np.asarray(Wo, np.float32)
    B, L, _ = q.shape
    M = k.shape[1]
    LT, MT = L // NT, M // NT

    import ml_dtypes
    bf = ml_dtypes.bfloat16
    Wq_p = Wq[_PERM]            # (1024, 1024) head-major rows
    Wk_p = Wk[_PERM]
    WoT_p = Wo[:, _PERM].T      # (1024 c, 1024 j)

    def pack_strips(x):         # x: (S, DIM) -> [S/NT, P, KC, NT]
        S = x.shape[0]
        xt = x.T.reshape(KC, P, S // NT, NT)
        return np.ascontiguousarray(xt.transpose(2, 1, 0, 3)).astype(bf)

    def pack_w(Whg):            # (CW, DIM) -> [P, KC, CW]
        wt = Whg.T.reshape(KC, P, CW)
        return np.ascontiguousarray(wt.transpose(1, 0, 2)).astype(bf)

    def pack_wo(WoThg):         # (CW, DIM) -> [P, G, DIM]
        wt = WoThg.reshape(G, P, DIM)
        return np.ascontiguousarray(wt.transpose(1, 0, 2)).astype(bf)

    qTs = [pack_strips(q[b]) for b in range(B)]
    kTs = [pack_strips(k[b]) for b in range(B)]
    wq_l = [pack_w(Wq_p[hg * CW:(hg + 1) * CW, :]) for hg in range(4)]
    wk_l = [pack_w(Wk_p[hg * CW:(hg + 1) * CW, :]) for hg in range(4)]
    wo_l = [pack_wo(WoT_p[hg * CW:(hg + 1) * CW, :]) for hg in range(4)]

    in_maps = []
    for core in range(8):
        b, hg = divmod(core, 4)
        in_maps.append({"qTs": qTs[b], "kTs": kTs[b], "wq": wq_l[hg],
                        "wk": wk_l[hg], "wo": wo_l[hg]})

    nc = _get_nc(L, M)
    trace = bool(int(os.environ.get("MHA_TRACE", "0")))
    res = run_bass_kernel_spmd(nc, in_maps, core_ids=list(range(8)), trace=trace)
    last_results = res
    last_exec_time_ns = res.exec_time_ns

    out = np.zeros((B, L, DIM), np.float32)
    for core in range(8):
        b = core // 4
        out[b] += np.asarray(res.results[core]["out"], dtype=np.float32)
    return out
